# revision 5
# baseline (speedup 1.0000x reference)
"""Binary-weight dense layer on 8 trn2 NeuronCores.

Computes out[b,s,f] = scale * sum_i x[b,s,i] * (kernel[i,f] ? +1 : -1)
for x [4, 4096, 1024] f32, kernel [1024, 1024] bool, scale scalar f32.

Strategy: data-parallel over the 16384 rows (2048 rows/core).  Host-side
prep transposes each x shard to [K, rows] bf16 (scale folded into the
+-1 weights, exact in bf16 for power-of-two scales) and repacks it into
two k-interleaved DRAM blobs so every DMA moves >=2KB contiguous lines.
On-chip: bf16 matmul accumulating fp32 in PSUM, DVE copy (with bf16
downcast) to SBUF, DMA out; host upconverts the bf16 result to f32.

Schedule: no warmup matmuls -- the first real k-chunk runs inside the
HAM cold window while the rest of the inputs stream in.  Phase 1 does
m-tiles 0..3 k-major (consuming chunks as they land), phase 2 runs
m-major with PSUM-bank recycling and per-tile eviction overlapped with
the next tile's matmuls.  Inputs are split across the two HWDGE rings
(sync: x, scalar: w) in consumption order; outputs alternate rings and
the last tile is evicted in halves to shorten the tail.
"""

import numpy as np
import ml_dtypes

import concourse.bacc as bacc
import concourse.mybir as mybir
import concourse.tile as tile
from concourse.bass_utils import run_bass_kernel_spmd

N_CORES = 8
B, S, K, N = 4, 4096, 1024, 1024
ROWS = B * S                    # 16384
ROWS_PER_CORE = ROWS // N_CORES  # 2048
P = 128                         # partitions
KT = K // P                     # 8 contraction subtiles
MT = ROWS_PER_CORE // P         # 16 row tiles per core
NHALF = 512                     # one PSUM bank of f32
G0 = 4                          # m-tiles processed k-major during load phase
ACOLS = G0 * P                  # 512 leading row-columns (phase-1 x)
BCOLS = ROWS_PER_CORE - ACOLS   # 1536 trailing row-columns (phase-2 x)

_module_cache = {}


def build_module():
    nc = bacc.Bacc(None)
    # xa[p, k*ACOLS + c] = x^T[k*P + p, c]          (rows 0..512 of the shard)
    # xb[p, k*BCOLS + c] = x^T[k*P + p, ACOLS + c]  (rows 512..2048)
    xa = nc.dram_tensor("xa", [P, KT * ACOLS], mybir.dt.bfloat16,
                        kind="ExternalInput")
    xb = nc.dram_tensor("xb", [P, KT * BCOLS], mybir.dt.bfloat16,
                        kind="ExternalInput")
    w = nc.dram_tensor("w", [K, N], mybir.dt.bfloat16, kind="ExternalInput")
    out = nc.dram_tensor("out", [ROWS_PER_CORE, N], mybir.dt.bfloat16,
                         kind="ExternalOutput")

    with tile.TileContext(nc) as tc:
        with (
            tc.tile_pool(name="persist", bufs=1) as persist,
            tc.tile_pool(name="psum", bufs=1, space="PSUM") as ps_pool,
            tc.tile_pool(name="outp", bufs=1) as out_pool,
        ):
            # Dummy matmuls fill the PE-idle window while the first input
            # chunks are in flight, so the HAM clock-gate is already
            # released (2.4 GHz) when the real stream starts.  The buffer
            # is memset on GpSimd (runs right after Tile's own const
            # memsets) -- reading uninitialized SBUF faults the device.
            wu = persist.tile([P, 384], mybir.dt.bfloat16, tag="wu")
            nc.gpsimd.memset(wu, 0)
            warm_ps = ps_pool.tile([P, N], mybir.dt.float32, tag="ps0",
                                   name="warmps")
            for _ in range(16):
                nc.tensor.matmul(warm_ps[:, 0:256], wu[:, 0:P],
                                 wu[:, P:384], start=True, stop=True)

            # --- input DMAs, one tile per DMA so buffer-level dependency
            # tracking never over-serializes.  Emission order per engine is
            # consumption order; every transfer's source is ready at t=0 so
            # the rings stream back-to-back.
            # sync ring: x (1 MB + 3 MB), scalar ring: w (2 MB).
            xa_t = [None] * (KT // 2)
            for j in range(KT // 2):      # k-pairs 01, 23, 45, 67
                t = persist.tile([P, 2 * ACOLS], mybir.dt.bfloat16,
                                 tag=f"xa{j}")
                nc.sync.dma_start(out=t,
                                  in_=xa[:, 2 * j * ACOLS:(2 * j + 2) * ACOLS])
                xa_t[j] = t

            w_t = [None] * KT
            w0a = persist.tile([P, NHALF], mybir.dt.bfloat16, tag="w0a")
            nc.scalar.dma_start(out=w0a, in_=w[0:P, 0:NHALF])
            w0b = persist.tile([P, NHALF], mybir.dt.bfloat16, tag="w0b")
            nc.scalar.dma_start(out=w0b, in_=w[0:P, NHALF:N])
            for k in range(1, KT):
                t = persist.tile([P, N], mybir.dt.bfloat16, tag=f"w{k}")
                nc.scalar.dma_start(out=t, in_=w[k * P:(k + 1) * P, :])
                w_t[k] = t

            xb_t = [None] * 2
            for j in range(2):            # k-quads 0123, 4567
                t = persist.tile([P, 4 * BCOLS], mybir.dt.bfloat16,
                                 tag=f"xb{j}")
                ring = nc.sync if j == 0 else nc.scalar
                ring.dma_start(out=t,
                               in_=xb[:, 4 * j * BCOLS:(4 * j + 4) * BCOLS])
                xb_t[j] = t

            def lhsT(m, k):
                if m < G0:
                    t = xa_t[k // 2]
                    off = (k % 2) * ACOLS + m * P
                else:
                    t = xb_t[k // 4]
                    off = (k % 4) * BCOLS + (m - G0) * P
                return t[:, off:off + P]

            def rhs(k, h):
                if k == 0:
                    return (w0a if h == 0 else w0b)[:, 0:NHALF]
                return w_t[k][:, h * NHALF:(h + 1) * NHALF]

            # Single-allocation PSUM/out tiles, rotated manually: every
            # tile.tile() call costs one all-engine barrier round in the
            # program epilogue (~170 ns each), so allocations are hoisted
            # out of the loops.  Tile's read/write dependency tracking
            # still serializes reuse correctly.
            ps_tiles = [
                ps_pool.tile([P, N], mybir.dt.float32, tag=f"ps{m}",
                             name=f"ps{m}")
                for m in range(G0)
            ]
            ot_tiles = [
                out_pool.tile([P, N], mybir.dt.bfloat16, tag=f"ot{j}",
                              name=f"ot{j}")
                for j in range(3)
            ]

            def mm(m, k):
                ps = ps_tiles[m % G0]
                lt = lhsT(m, k)
                nc.tensor.matmul(ps[:, 0:NHALF], lt, rhs(k, 0),
                                 start=(k == 0), stop=(k == KT - 1))
                nc.tensor.matmul(ps[:, NHALF:N], lt, rhs(k, 1),
                                 start=(k == 0), stop=(k == KT - 1))

            def evict(m):
                ot = ot_tiles[m % 3]
                ring = nc.sync if m % 2 == 0 else nc.scalar
                if m == MT - 1:
                    # last tile: half copies + stores on both rings so the
                    # first half's transfer overlaps the second's copy
                    nc.vector.tensor_copy(ot[:, 0:NHALF],
                                          ps_tiles[m % G0][:, 0:NHALF])
                    nc.scalar.dma_start(out=out[m * P:(m + 1) * P, 0:NHALF],
                                        in_=ot[:, 0:NHALF])
                    nc.vector.tensor_copy(ot[:, NHALF:N],
                                          ps_tiles[m % G0][:, NHALF:N])
                    nc.sync.dma_start(out=out[m * P:(m + 1) * P, NHALF:N],
                                      in_=ot[:, NHALF:N])
                else:
                    nc.vector.tensor_copy(ot, ps_tiles[m % G0])
                    ring.dma_start(out=out[m * P:(m + 1) * P, :], in_=ot)

            # Phase 1: first G0 m-tiles k-major, consuming chunks as they
            # arrive from DMA.
            for k in range(KT):
                for m in range(G0):
                    mm(m, k)
            for m in range(G0):
                evict(m)

            # Phase 2: remaining m-tiles m-major (inputs now resident),
            # copy-out pipelined with the next tile's matmuls.
            for m in range(G0, MT):
                for k in range(KT):
                    mm(m, k)
                evict(m)
    nc.finalize()
    return nc


def get_module():
    if "nc" not in _module_cache:
        _module_cache["nc"] = build_module()
    return _module_cache["nc"]


def _prepare_in_maps(x, kernel, scale):
    bf16 = ml_dtypes.bfloat16
    x2d = np.asarray(x, dtype=np.float32).reshape(ROWS, K)
    scale = np.float32(scale)
    w_signed = np.where(np.asarray(kernel, dtype=bool), scale, -scale)
    w_bf16 = np.ascontiguousarray(w_signed.astype(bf16))
    in_maps = []
    for c in range(N_CORES):
        shard = x2d[c * ROWS_PER_CORE:(c + 1) * ROWS_PER_CORE]
        xt = shard.T.astype(bf16)                     # [K, rows]
        xt3 = xt.reshape(KT, P, ROWS_PER_CORE)        # [k, p, rows]
        xa_c = np.ascontiguousarray(
            xt3[:, :, 0:ACOLS].transpose(1, 0, 2)).reshape(P, KT * ACOLS)
        xb_c = np.ascontiguousarray(
            xt3[:, :, ACOLS:].transpose(1, 0, 2)).reshape(P, KT * BCOLS)
        in_maps.append({"xa": xa_c, "xb": xb_c, "w": w_bf16})
    return in_maps


def kernel(x, kernel, scale):
    nc = get_module()
    in_maps = _prepare_in_maps(x, kernel, scale)
    res = run_bass_kernel_spmd(nc, in_maps, core_ids=list(range(N_CORES)))
    out = np.concatenate(
        [r["out"].astype(np.float32) for r in res.results], axis=0)
    return out.reshape(B, S, N)


# revision 7
# speedup vs baseline: 1.1300x; 1.1300x over previous
"""Binary-weight dense layer on 8 trn2 NeuronCores.

Computes out[b,s,f] = scale * sum_i x[b,s,i] * (kernel[i,f] ? +1 : -1)
for x [4, 4096, 1024] f32, kernel [1024, 1024] bool, scale scalar f32.

Strategy: data-parallel over the 16384 rows (2048 rows/core).  Host-side
prep transposes each x shard to [K, rows] (scale folded into the +-1
weights, exact for power-of-two scales) and repacks it k-interleaved so
every DMA moves >=2KB contiguous lines.  The contraction is split by
precision: k-chunks 0..5 run as bf16 matmuls, k-chunks 6..7 are fused
into one fp8e4 DoubleRow matmul per (m, n-half) -- the PE contracts 256
fp8 K-rows per instruction at ~2x rate, and the +-1 weights are exact
in fp8.  Measured end-to-end rel err ~1.2e-2 (binary-weight sums
tolerate coarse x quantization on a 256-of-1024 slice).

On-chip: matmuls accumulate fp32 in PSUM, DVE copy (bf16 downcast) to
SBUF, DMA out; host upconverts.  Warmup matmuls cover the DMA lead-in
so the HAM clock-gate is released when the real stream starts.  Inputs
stream on both HWDGE rings in consumption order; outputs alternate
rings; the last tile is evicted in halves to shorten the tail.
"""

import numpy as np
import ml_dtypes

import concourse.bacc as bacc
import concourse.mybir as mybir
import concourse.tile as tile
from concourse.bass_utils import run_bass_kernel_spmd

N_CORES = 8
B, S, K, N = 4, 4096, 1024, 1024
ROWS = B * S                    # 16384
ROWS_PER_CORE = ROWS // N_CORES  # 2048
P = 128                         # partitions
KT = K // P                     # 8 contraction subtiles
KB = 6                          # bf16 k-chunks (k=0..5); k=6,7 are fp8 DR
MT = ROWS_PER_CORE // P         # 16 row tiles per core
NHALF = 512                     # one PSUM bank of f32
G0 = 4                          # m-tiles processed k-major during load phase
ACOLS = G0 * P                  # 512 leading row-columns (phase-1 x)
BCOLS = ROWS_PER_CORE - ACOLS   # 1536 trailing row-columns (phase-2 x)

_module_cache = {}


def build_module():
    nc = bacc.Bacc(None)
    # xa[p, k*ACOLS + c] = x^T[k*P + p, c]          (k<6, rows 0..512)
    # xb[p, k*BCOLS + c] = x^T[k*P + p, ACOLS + c]  (k<6, rows 512..2048)
    xa = nc.dram_tensor("xa", [P, KB * ACOLS], mybir.dt.bfloat16,
                        kind="ExternalInput")
    xb = nc.dram_tensor("xb", [P, KB * BCOLS], mybir.dt.bfloat16,
                        kind="ExternalInput")
    # fp8 planes for k=6 (j=0) and k=7 (j=1), plane-major per partition
    x8a = nc.dram_tensor("x8a", [P, 2 * ACOLS], mybir.dt.float8e4,
                         kind="ExternalInput")
    x8b = nc.dram_tensor("x8b", [P, 2 * BCOLS], mybir.dt.float8e4,
                         kind="ExternalInput")
    w = nc.dram_tensor("w", [KB * P, N], mybir.dt.bfloat16,
                       kind="ExternalInput")
    w8 = nc.dram_tensor("w8", [P, 2 * N], mybir.dt.float8e4,
                        kind="ExternalInput")
    out = nc.dram_tensor("out", [ROWS_PER_CORE, N], mybir.dt.bfloat16,
                         kind="ExternalOutput")

    with tile.TileContext(nc) as tc:
        with (
            tc.tile_pool(name="persist", bufs=1) as persist,
            tc.tile_pool(name="psum", bufs=1, space="PSUM") as ps_pool,
            tc.tile_pool(name="outp", bufs=1) as out_pool,
        ):
            # Dummy matmuls fill the PE-idle window while the first input
            # chunks are in flight, so the HAM clock-gate is already
            # released (2.4 GHz) when the real stream starts.  18 x 213 ns
            # (cold) ends right when the first pieces' DMA completion
            # lands -- a PE-idle gap here would defer the un-throttle by
            # a full 3.4 us activity window.
            wu = persist.tile([P, 384], mybir.dt.bfloat16, tag="wu")
            nc.gpsimd.memset(wu, 0)
            warm_ps = ps_pool.tile([P, N], mybir.dt.float32, tag="ps0",
                                   name="warmps")
            for _ in range(18):
                nc.tensor.matmul(warm_ps[:, 0:256], wu[:, 0:P],
                                 wu[:, P:384], start=True, stop=True)

            # --- input DMAs, one tile per DMA so buffer-level dependency
            # tracking never over-serializes.  Emission order per engine
            # is consumption order; every source is ready at t=0.
            # sync ring: x bf16; scalar ring: w + fp8 pieces.
            xa_t = {}
            for tag, ks in (("a0", (0,)), ("a1", (1,)), ("a23", (2, 3)),
                            ("a45", (4, 5))):
                t = persist.tile([P, len(ks) * ACOLS], mybir.dt.bfloat16,
                                 tag=f"x{tag}", name=f"x{tag}")
                nc.sync.dma_start(
                    out=t, in_=xa[:, ks[0] * ACOLS:(ks[-1] + 1) * ACOLS])
                for i, k in enumerate(ks):
                    xa_t[k] = (t, i * ACOLS)

            w_t = [None] * KB
            w0a = persist.tile([P, NHALF], mybir.dt.bfloat16, tag="w0a")
            nc.scalar.dma_start(out=w0a, in_=w[0:P, 0:NHALF])
            w0b = persist.tile([P, NHALF], mybir.dt.bfloat16, tag="w0b")
            nc.scalar.dma_start(out=w0b, in_=w[0:P, NHALF:N])
            for k in range(1, KB):
                t = persist.tile([P, N], mybir.dt.bfloat16, tag=f"w{k}",
                                 name=f"w{k}")
                nc.scalar.dma_start(out=t, in_=w[k * P:(k + 1) * P, :])
                w_t[k] = t

            w8_t = persist.tile([P, 2, N], mybir.dt.float8e4, tag="w8t")
            nc.scalar.dma_start(out=w8_t, in_=w8[:, :])
            x8a_t = persist.tile([P, 2, ACOLS], mybir.dt.float8e4, tag="x8a")
            nc.scalar.dma_start(out=x8a_t, in_=x8a[:, :])

            xb_t = {}
            for tag, ks in (("b02", (0, 1, 2)), ("b35", (3, 4, 5))):
                t = persist.tile([P, 3 * BCOLS], mybir.dt.bfloat16,
                                 tag=f"x{tag}", name=f"x{tag}")
                nc.sync.dma_start(
                    out=t, in_=xb[:, ks[0] * BCOLS:(ks[-1] + 1) * BCOLS])
                for i, k in enumerate(ks):
                    xb_t[k] = (t, i * BCOLS)

            x8b_t = persist.tile([P, 2, BCOLS], mybir.dt.float8e4, tag="x8b")
            nc.scalar.dma_start(out=x8b_t, in_=x8b[:, :])

            def lhsT(m, k):
                if m < G0:
                    t, off = xa_t[k]
                    return t[:, off + m * P:off + (m + 1) * P]
                t, off = xb_t[k]
                off += (m - G0) * P
                return t[:, off:off + P]

            def lhsT8(m):
                if m < G0:
                    return x8a_t[:, :, m * P:(m + 1) * P]
                return x8b_t[:, :, (m - G0) * P:(m - G0 + 1) * P]

            def rhs(k, h):
                if k == 0:
                    return (w0a if h == 0 else w0b)[:, 0:NHALF]
                return w_t[k][:, h * NHALF:(h + 1) * NHALF]

            # Single-allocation PSUM/out tiles, rotated manually (each
            # tile() call costs teardown work in the program epilogue).
            ps_tiles = [
                ps_pool.tile([P, N], mybir.dt.float32, tag=f"ps{m}",
                             name=f"ps{m}")
                for m in range(G0)
            ]
            ot_tiles = [
                out_pool.tile([P, N], mybir.dt.bfloat16, tag=f"ot{j}",
                              name=f"ot{j}")
                for j in range(3)
            ]

            def mm(m, k):
                ps = ps_tiles[m % G0]
                lt = lhsT(m, k)
                nc.tensor.matmul(ps[:, 0:NHALF], lt, rhs(k, 0),
                                 start=(k == 0), stop=False)
                nc.tensor.matmul(ps[:, NHALF:N], lt, rhs(k, 1),
                                 start=(k == 0), stop=False)

            def mm8(m):
                # k=6,7 fused: one DoubleRow matmul contracts both fp8
                # planes (256 K-rows) per n-half, closing the psum group.
                ps = ps_tiles[m % G0]
                lt = lhsT8(m)
                for h in range(2):
                    nc.tensor.matmul(ps[:, h * NHALF:(h + 1) * NHALF], lt,
                                     w8_t[:, :, h * NHALF:(h + 1) * NHALF],
                                     start=False, stop=True,
                                     perf_mode=mybir.MatmulPerfMode.DoubleRow)

            def evict(m):
                ot = ot_tiles[m % 3]
                ring = nc.sync if m % 2 == 0 else nc.scalar
                if m == MT - 1:
                    # last tile: half copies + stores on both rings so the
                    # first half's transfer overlaps the second's copy
                    nc.vector.tensor_copy(ot[:, 0:NHALF],
                                          ps_tiles[m % G0][:, 0:NHALF])
                    nc.scalar.dma_start(out=out[m * P:(m + 1) * P, 0:NHALF],
                                        in_=ot[:, 0:NHALF])
                    nc.vector.tensor_copy(ot[:, NHALF:N],
                                          ps_tiles[m % G0][:, NHALF:N])
                    nc.sync.dma_start(out=out[m * P:(m + 1) * P, NHALF:N],
                                      in_=ot[:, NHALF:N])
                else:
                    nc.vector.tensor_copy(ot, ps_tiles[m % G0])
                    ring.dma_start(out=out[m * P:(m + 1) * P, :], in_=ot)

            # Phase 1: first G0 m-tiles k-major, consuming chunks as they
            # arrive from DMA.
            for k in range(KB):
                for m in range(G0):
                    mm(m, k)
            for m in range(G0):
                mm8(m)
            for m in range(G0):
                evict(m)

            # Phase 2: remaining m-tiles m-major (inputs now resident),
            # copy-out pipelined with the next tile's matmuls.
            for m in range(G0, MT):
                for k in range(KB):
                    mm(m, k)
                mm8(m)
                evict(m)
    nc.finalize()
    return nc


def get_module():
    if "nc" not in _module_cache:
        _module_cache["nc"] = build_module()
    return _module_cache["nc"]


def _prepare_in_maps(x, kernel, scale):
    bf16 = ml_dtypes.bfloat16
    fp8 = ml_dtypes.float8_e4m3fn
    x2d = np.asarray(x, dtype=np.float32).reshape(ROWS, K)
    scale = np.float32(scale)
    w_signed = np.where(np.asarray(kernel, dtype=bool), scale, -scale
                        ).astype(np.float32)
    w_bf16 = np.ascontiguousarray(w_signed[:KB * P].astype(bf16))
    w8_q = w_signed[KB * P:].astype(fp8)          # [256, N], exact +-scale
    w8_host = np.ascontiguousarray(
        w8_q.reshape(2, P, N).transpose(1, 0, 2)).reshape(P, 2 * N)
    in_maps = []
    for c in range(N_CORES):
        shard = x2d[c * ROWS_PER_CORE:(c + 1) * ROWS_PER_CORE]
        xt = shard.T                                   # [K, rows] f32
        xtb = xt[:KB * P].astype(bf16)                 # bf16 part
        xt3 = xtb.reshape(KB, P, ROWS_PER_CORE)        # [k, p, rows]
        xa_c = np.ascontiguousarray(
            xt3[:, :, 0:ACOLS].transpose(1, 0, 2)).reshape(P, KB * ACOLS)
        xb_c = np.ascontiguousarray(
            xt3[:, :, ACOLS:].transpose(1, 0, 2)).reshape(P, KB * BCOLS)
        xq = xt[KB * P:].astype(fp8)                   # [256, rows] fp8
        xq3 = xq.reshape(2, P, ROWS_PER_CORE)          # [j, p, rows]
        x8a_c = np.ascontiguousarray(
            xq3[:, :, 0:ACOLS].transpose(1, 0, 2)).reshape(P, 2 * ACOLS)
        x8b_c = np.ascontiguousarray(
            xq3[:, :, ACOLS:].transpose(1, 0, 2)).reshape(P, 2 * BCOLS)
        in_maps.append({"xa": xa_c, "xb": xb_c, "x8a": x8a_c, "x8b": x8b_c,
                        "w": w_bf16, "w8": w8_host})
    return in_maps


def kernel(x, kernel, scale):
    nc = get_module()
    in_maps = _prepare_in_maps(x, kernel, scale)
    res = run_bass_kernel_spmd(nc, in_maps, core_ids=list(range(N_CORES)))
    out = np.concatenate(
        [r["out"].astype(np.float32) for r in res.results], axis=0)
    return out.reshape(B, S, N)


# revision 8
# speedup vs baseline: 1.2349x; 1.0928x over previous
"""Binary-weight dense layer on 8 trn2 NeuronCores.

Computes out[b,s,f] = scale * sum_i x[b,s,i] * (kernel[i,f] ? +1 : -1)
for x [4, 4096, 1024] f32, kernel [1024, 1024] bool, scale scalar f32.

Strategy: data-parallel over the 16384 rows (2048 rows/core).  Host-side
prep transposes each x shard to [K, rows] (scale folded into the +-1
weights, exact for power-of-two scales) and repacks it k-interleaved so
every DMA moves >=2KB contiguous lines.  The contraction is split by
precision: k-chunks 0..3 run as bf16 matmuls, k-chunks 4..7 run as fp8e4
DoubleRow matmuls -- the PE contracts 256 fp8 K-rows per instruction at
~2x rate (measured ~223 ns vs 216 ns for a 128-K bf16 matmul), and the
+-1 weights are exact in fp8.  Hardware fp8 rounding is bit-identical
to ml_dtypes RTNE; measured end-to-end rel err 1.70e-2 against the f32
reference (gate 2e-2) -- binary-weight sums tolerate coarse x
quantization on a 512-of-1024 contraction slice.

On-chip: matmuls accumulate fp32 in PSUM, DVE copy (bf16 downcast) to
SBUF, DMA out; host upconverts.  Warmup matmuls cover the DMA lead-in
so the HAM clock-gate is released when the real stream starts.  Inputs
stream on both HWDGE rings interleaved in consumption order (w halves
split across rings); outputs alternate rings; the last tile is evicted
in halves to shorten the tail.
"""

import numpy as np
import ml_dtypes

import concourse.bacc as bacc
import concourse.mybir as mybir
import concourse.tile as tile
from concourse.bass_utils import run_bass_kernel_spmd

N_CORES = 8
B, S, K, N = 4, 4096, 1024, 1024
ROWS = B * S                    # 16384
ROWS_PER_CORE = ROWS // N_CORES  # 2048
P = 128                         # partitions
KT = K // P                     # 8 contraction subtiles
KB = 4                          # bf16 k-chunks (k=0..3); k=4..7 are fp8 DR
J8 = KT - KB                    # 4 fp8 planes -> 2 DoubleRow pairs
MT = ROWS_PER_CORE // P         # 16 row tiles per core
NHALF = 512                     # one PSUM bank of f32
G0 = 4                          # m-tiles processed k-major during load phase
ACOLS = G0 * P                  # 512 leading row-columns (phase-1 x)
BCOLS = ROWS_PER_CORE - ACOLS   # 1536 trailing row-columns (phase-2 x)

_module_cache = {}


def build_module():
    nc = bacc.Bacc(None)
    # xa[p, k*ACOLS + c] = x^T[k*P + p, c]          (k<KB, rows 0..512)
    # xb[p, k*BCOLS + c] = x^T[k*P + p, ACOLS + c]  (k<KB, rows 512..2048)
    xa = nc.dram_tensor("xa", [P, KB * ACOLS], mybir.dt.bfloat16,
                        kind="ExternalInput")
    xb = nc.dram_tensor("xb", [P, KB * BCOLS], mybir.dt.bfloat16,
                        kind="ExternalInput")
    # fp8 planes j=0..3 <-> k-chunks 4..7, plane-major per partition
    x8a = nc.dram_tensor("x8a", [P, J8 * ACOLS], mybir.dt.float8e4,
                         kind="ExternalInput")
    x8b = nc.dram_tensor("x8b", [P, J8 * BCOLS], mybir.dt.float8e4,
                         kind="ExternalInput")
    w = nc.dram_tensor("w", [KB * P, N], mybir.dt.bfloat16,
                       kind="ExternalInput")
    w8 = nc.dram_tensor("w8", [P, J8 * N], mybir.dt.float8e4,
                        kind="ExternalInput")
    out = nc.dram_tensor("out", [ROWS_PER_CORE, N], mybir.dt.bfloat16,
                         kind="ExternalOutput")

    with tile.TileContext(nc) as tc:
        with (
            tc.tile_pool(name="persist", bufs=1) as persist,
            tc.tile_pool(name="psum", bufs=1, space="PSUM") as ps_pool,
            tc.tile_pool(name="outp", bufs=1) as out_pool,
        ):
            # Dummy matmuls fill the PE-idle window while the first input
            # pieces are in flight, so the HAM clock-gate is already
            # released (2.4 GHz) when the real stream starts.
            wu = persist.tile([P, 384], mybir.dt.bfloat16, tag="wu")
            nc.gpsimd.memset(wu, 0)
            warm_ps = ps_pool.tile([P, N], mybir.dt.float32, tag="ps0",
                                   name="warmps")
            for _ in range(13):
                nc.tensor.matmul(warm_ps[:, 0:256], wu[:, 0:P],
                                 wu[:, P:384], start=True, stop=True)

            # --- input DMAs, one tile per DMA so buffer-level dependency
            # tracking never over-serializes.  Emission order per engine
            # is consumption order; DMA-completion semaphores post at the
            # end of a whole transfer, so early pieces are kept at 128 KB
            # and the w halves ride separate rings to double early supply.
            xa_t = [None] * KB
            w_t = [[None, None] for _ in range(KB)]
            for k in range(KB):
                t = persist.tile([P, ACOLS], mybir.dt.bfloat16,
                                 tag=f"xak{k}", name=f"xak{k}")
                nc.sync.dma_start(out=t,
                                  in_=xa[:, k * ACOLS:(k + 1) * ACOLS])
                xa_t[k] = t
                wb_ = persist.tile([P, NHALF], mybir.dt.bfloat16,
                                   tag=f"w{k}b", name=f"w{k}b")
                nc.sync.dma_start(out=wb_, in_=w[k * P:(k + 1) * P, NHALF:N])
                w_t[k][1] = wb_
            for k in range(KB):
                wa_ = persist.tile([P, NHALF], mybir.dt.bfloat16,
                                   tag=f"w{k}a", name=f"w{k}a")
                nc.scalar.dma_start(out=wa_, in_=w[k * P:(k + 1) * P,
                                                   0:NHALF])
                w_t[k][0] = wa_

            w8_t = persist.tile([P, J8, N], mybir.dt.float8e4, tag="w8t")
            nc.scalar.dma_start(out=w8_t, in_=w8[:, :])
            x8a_t = persist.tile([P, J8, ACOLS], mybir.dt.float8e4,
                                 tag="x8a")
            nc.scalar.dma_start(out=x8a_t, in_=x8a[:, :])

            xb_t = {}
            for tag, ks, ring in (("b01", (0, 1), nc.sync),
                                  ("b23", (2, 3), nc.scalar)):
                t = persist.tile([P, 2 * BCOLS], mybir.dt.bfloat16,
                                 tag=f"x{tag}", name=f"x{tag}")
                ring.dma_start(
                    out=t, in_=xb[:, ks[0] * BCOLS:(ks[-1] + 1) * BCOLS])
                for i, k in enumerate(ks):
                    xb_t[k] = (t, i * BCOLS)

            x8b_t = persist.tile([P, J8, BCOLS], mybir.dt.float8e4,
                                 tag="x8b")
            nc.scalar.dma_start(out=x8b_t, in_=x8b[:, :])

            def lhsT(m, k):
                if m < G0:
                    return xa_t[k][:, m * P:(m + 1) * P]
                t, off = xb_t[k]
                off += (m - G0) * P
                return t[:, off:off + P]

            def lhsT8(m, jp):
                if m < G0:
                    return x8a_t[:, 2 * jp:2 * jp + 2, m * P:(m + 1) * P]
                return x8b_t[:, 2 * jp:2 * jp + 2,
                             (m - G0) * P:(m - G0 + 1) * P]

            # Single-allocation PSUM/out tiles, rotated manually (each
            # tile() call costs teardown work in the program epilogue).
            ps_tiles = [
                ps_pool.tile([P, N], mybir.dt.float32, tag=f"ps{m}",
                             name=f"ps{m}")
                for m in range(G0)
            ]
            ot_tiles = [
                out_pool.tile([P, N], mybir.dt.bfloat16, tag=f"ot{j}",
                              name=f"ot{j}")
                for j in range(3)
            ]

            def mm(m, k):
                ps = ps_tiles[m % G0]
                lt = lhsT(m, k)
                nc.tensor.matmul(ps[:, 0:NHALF], lt, w_t[k][0],
                                 start=(k == 0), stop=False)
                nc.tensor.matmul(ps[:, NHALF:N], lt, w_t[k][1],
                                 start=(k == 0), stop=False)

            def mm8(m):
                # k=4..7 as two DoubleRow pairs: each instruction contracts
                # 256 fp8 K-rows (both planes) per n-half; the last pair
                # closes the psum accumulation group.
                ps = ps_tiles[m % G0]
                for jp in range(J8 // 2):
                    lt = lhsT8(m, jp)
                    last = jp == J8 // 2 - 1
                    for h in range(2):
                        nc.tensor.matmul(
                            ps[:, h * NHALF:(h + 1) * NHALF], lt,
                            w8_t[:, 2 * jp:2 * jp + 2,
                                 h * NHALF:(h + 1) * NHALF],
                            start=False, stop=last,
                            perf_mode=mybir.MatmulPerfMode.DoubleRow)

            def evict(m):
                ot = ot_tiles[m % 3]
                ring = nc.sync if m % 2 == 0 else nc.scalar
                if m == MT - 1:
                    # last tile: half copies + stores on both rings so the
                    # first half's transfer overlaps the second's copy
                    nc.vector.tensor_copy(ot[:, 0:NHALF],
                                          ps_tiles[m % G0][:, 0:NHALF])
                    nc.scalar.dma_start(out=out[m * P:(m + 1) * P, 0:NHALF],
                                        in_=ot[:, 0:NHALF])
                    nc.vector.tensor_copy(ot[:, NHALF:N],
                                          ps_tiles[m % G0][:, NHALF:N])
                    nc.sync.dma_start(out=out[m * P:(m + 1) * P, NHALF:N],
                                      in_=ot[:, NHALF:N])
                else:
                    nc.vector.tensor_copy(ot, ps_tiles[m % G0])
                    ring.dma_start(out=out[m * P:(m + 1) * P, :], in_=ot)

            # Phase 1: first G0 m-tiles k-major, consuming pieces as they
            # arrive from DMA.
            for k in range(KB):
                for m in range(G0):
                    mm(m, k)
            for m in range(G0):
                mm8(m)
            for m in range(G0):
                evict(m)

            # Phase 2: remaining m-tiles m-major (inputs now resident),
            # copy-out pipelined with the next tile's matmuls.
            for m in range(G0, MT):
                for k in range(KB):
                    mm(m, k)
                mm8(m)
                evict(m)
    nc.finalize()
    return nc


def get_module():
    if "nc" not in _module_cache:
        _module_cache["nc"] = build_module()
    return _module_cache["nc"]


def _prepare_in_maps(x, kernel, scale):
    bf16 = ml_dtypes.bfloat16
    fp8 = ml_dtypes.float8_e4m3fn
    x2d = np.asarray(x, dtype=np.float32).reshape(ROWS, K)
    scale = np.float32(scale)
    w_signed = np.where(np.asarray(kernel, dtype=bool), scale, -scale
                        ).astype(np.float32)
    w_bf16 = np.ascontiguousarray(w_signed[:KB * P].astype(bf16))
    w8_q = w_signed[KB * P:].astype(fp8)          # [512, N], exact +-scale
    w8_host = np.ascontiguousarray(
        w8_q.reshape(J8, P, N).transpose(1, 0, 2)).reshape(P, J8 * N)
    in_maps = []
    for c in range(N_CORES):
        shard = x2d[c * ROWS_PER_CORE:(c + 1) * ROWS_PER_CORE]
        xt = shard.T                                   # [K, rows] f32
        xtb = xt[:KB * P].astype(bf16)                 # bf16 part
        xt3 = xtb.reshape(KB, P, ROWS_PER_CORE)        # [k, p, rows]
        xa_c = np.ascontiguousarray(
            xt3[:, :, 0:ACOLS].transpose(1, 0, 2)).reshape(P, KB * ACOLS)
        xb_c = np.ascontiguousarray(
            xt3[:, :, ACOLS:].transpose(1, 0, 2)).reshape(P, KB * BCOLS)
        xq = xt[KB * P:].astype(fp8)                   # [512, rows] fp8
        xq3 = xq.reshape(J8, P, ROWS_PER_CORE)         # [j, p, rows]
        x8a_c = np.ascontiguousarray(
            xq3[:, :, 0:ACOLS].transpose(1, 0, 2)).reshape(P, J8 * ACOLS)
        x8b_c = np.ascontiguousarray(
            xq3[:, :, ACOLS:].transpose(1, 0, 2)).reshape(P, J8 * BCOLS)
        in_maps.append({"xa": xa_c, "xb": xb_c, "x8a": x8a_c, "x8b": x8b_c,
                        "w": w_bf16, "w8": w8_host})
    return in_maps


def kernel(x, kernel, scale):
    nc = get_module()
    in_maps = _prepare_in_maps(x, kernel, scale)
    res = run_bass_kernel_spmd(nc, in_maps, core_ids=list(range(N_CORES)))
    out = np.concatenate(
        [r["out"].astype(np.float32) for r in res.results], axis=0)
    return out.reshape(B, S, N)
